# revision 14
# baseline (speedup 1.0000x reference)
"""MLA self-attention block (eval mode) on 8 Trainium2 NeuronCores.

Sharding: tensor-parallel over heads (16 heads -> 2 per core), batch kept
whole per core.  The kv-latent projection is T-sharded: each core computes
kv_latent for its own T/8 = 256 token slice (from a per-core xT_kv input
slice), and an on-device AllGather replicates the full [B,T,L] latent to
every core.  Each core computes a partial output through its two heads'
slice of w_o; the host sums the 8 bf16 partials in f32.

Phase structure (the AllGather takes ~55us wall, so it is issued first and
hidden under collective-independent work):
  phase 1: kv partial (64 MMs) -> bounce -> AllGather -> kvT readback
  phase 2: q projections for all 8 (b, t-chunk) pairs (~100us of PE work,
           needs only xT and w_q) while the collective completes
  phase 3: keff + v for all chunks, then attention per chunk in
           DESCENDING j order (tail ends on a 4-step j=0 chunk), with the
           previous chunk's output projection as PE filler

Math (per core, heads h0=2c, h1=2c+1):
  kv_part [L,256]  = w_dkv^T @ xT_kv      (1/8 of the kv FLOPs)
  kvT [L,T]        = AllGather(kv_part)   (bf16, 512KB -> 4MB)
  qT_h [S,T]       = w_q[:,h]^T @ xT      (un-absorbed)
  k_effT [S,T]     = w_uk_h^T @ kvT       (w_uk absorbed into KEYS)
  attT [s,q]       = k_effT^T-tile @ qT   (causal: only s <= q tiles)
  probs            = exp(scale*attT) * tri_mask
  yT [S,q]         = v^T-tiles @ probs    (accumulated over s-tiles)
  den [1,q]        = ones^T @ probs
  out_partial      = (yT/den)^T @ w_o_rows (2 heads accumulated), bf16

All matmuls in bf16 (f32 PSUM accumulate).  xT chunk loads are single
2MB DMAs on the Sync HWDGE queue; wdkv/wq ride the Scalar HWDGE queue
which afterwards carries the per-chunk output stores; the collective
bounce + readback and late weights ride the GpSimd SWDGE queue.
"""

import sys
import os

sys.path.insert(0, "/opt/trn_rl_repo")

import numpy as np
from contextlib import ExitStack

import concourse.bass as bass
import concourse.tile as tile
from concourse import bacc, mybir
from concourse import bass_utils

F32 = mybir.dt.float32
BF16 = mybir.dt.bfloat16

B, T, C = 2, 2048, 2048
H, S, L = 16, 128, 512
NCORES = 8
HPC = H // NCORES  # 2 heads per core
NT = T // 512  # 4 t-chunks of 512
TKV = T // NCORES  # 256-token kv slice per core
SCALE = float(1.0 / np.sqrt(np.float32(C)))

_CACHE = {}


def _build():
    nc = bacc.Bacc("TRN2", target_bir_lowering=False, debug=False, num_devices=NCORES)

    xt_ap = nc.dram_tensor("xT", [B, C, T], BF16, kind="ExternalInput").ap()
    xkv_ap = nc.dram_tensor("xT_kv", [B, C, TKV], BF16, kind="ExternalInput").ap()
    w_dkv = nc.dram_tensor("w_dkv", [C, L], BF16, kind="ExternalInput").ap()
    w_q_sl = nc.dram_tensor("w_q_sl", [C, HPC * S], BF16, kind="ExternalInput").ap()
    w_ukT_sl = nc.dram_tensor("w_ukT_sl", [L, HPC * S], BF16, kind="ExternalInput").ap()
    w_uv_sl = nc.dram_tensor("w_uv_sl", [L, HPC * S], BF16, kind="ExternalInput").ap()
    w_o_sl = nc.dram_tensor("w_o_sl", [HPC * S, C], BF16, kind="ExternalInput").ap()
    tri_d = nc.dram_tensor("tri", [128, 128], BF16, kind="ExternalInput").ap()
    onesc_d = nc.dram_tensor("ones_col", [128, 1], BF16, kind="ExternalInput").ap()
    onesr_d = nc.dram_tensor("ones_row", [1, 128], BF16, kind="ExternalInput").ap()
    out_ap = nc.dram_tensor("out", [B, T, C], BF16, kind="ExternalOutput").ap()

    w_dkv_r = w_dkv.rearrange("(cc p) l -> p cc l", p=128)
    w_q_r = w_q_sl.rearrange("(cc p) f -> p cc f", p=128)

    with tile.TileContext(nc) as tc:
        with ExitStack() as ctx:
            wpool = ctx.enter_context(tc.tile_pool(name="w", bufs=1))
            pers = ctx.enter_context(tc.tile_pool(name="pers", bufs=1))
            psA = ctx.enter_context(tc.tile_pool(name="psA", bufs=4, space="PSUM"))
            psB = ctx.enter_context(tc.tile_pool(name="psB", bufs=3, space="PSUM"))
            psC = ctx.enter_context(tc.tile_pool(name="psC", bufs=1, space="PSUM"))
            dram = ctx.enter_context(tc.tile_pool(name="dram", bufs=1, space="DRAM"))

            # ---- weight loads.  scalar HWDGE: wdkv (kv partial needs it
            #      first), then wq.  sync HWDGE: xkv then the xtc stream.
            #      gpsimd SWDGE: late-phase weights.  Phase-1-only tiles
            #      (wdkv, xkv, kvloc) live in a pool released afterwards. ----
            ph1 = tc.alloc_tile_pool(name="ph1", bufs=1)
            wdkv_t = []
            for cc in range(16):
                wd = ph1.tile([128, L], BF16, tag=f"wdkv{cc}", name=f"wdkv{cc}")
                nc.scalar.dma_start(wd[:], w_dkv_r[:, cc, :])
                wdkv_t.append(wd)
            wq_t = []
            for cc in range(16):
                wqc = wpool.tile([128, HPC * S], BF16, tag=f"wq{cc}", name=f"wq{cc}")
                nc.scalar.dma_start(wqc[:], w_q_r[:, cc, :])
                wq_t.append(wqc)
            xkvt = ph1.tile([128, 16, B, TKV], BF16, tag="xkv", name="xkv")
            for b_ in range(B):
                nc.sync.dma_start(
                    xkvt[:, :, b_, :],
                    xkv_ap[b_].rearrange("(cc p) t -> p cc t", p=128),
                )

            wukT = wpool.tile([128, 4, HPC * S], BF16, tag="wukT", name="wukT")
            nc.gpsimd.dma_start(
                wukT[:], w_ukT_sl.rearrange("(lc p) f -> p lc f", p=128)
            )
            wuv = wpool.tile([128, 4, HPC * S], BF16, tag="wuv", name="wuv")
            nc.gpsimd.dma_start(wuv[:], w_uv_sl.rearrange("(lc p) f -> p lc f", p=128))
            wo = wpool.tile([128, HPC, C], BF16, tag="wo", name="wo")
            nc.gpsimd.dma_start(wo[:], w_o_sl.rearrange("(h p) f -> p h f", p=128))
            tri = wpool.tile([128, 128], BF16, tag="tri", name="tri")
            nc.gpsimd.dma_start(tri[:], tri_d)
            onesc = wpool.tile([128, 1], BF16, tag="onesc", name="onesc")
            nc.gpsimd.dma_start(onesc[:], onesc_d)
            onesr = wpool.tile([1, 128], BF16, tag="onesr", name="onesr")
            nc.gpsimd.dma_start(onesr[:], onesr_d)

            # ======== phase 1: kv partial + AllGather ========
            kvp = [
                psA.tile([128, B * TKV], F32, tag="acc4", name=f"kvp{lc}")
                for lc in range(4)
            ]
            for cc in range(16):
                for lc in range(4):
                    nc.tensor.matmul(
                        kvp[lc][:],
                        wdkv_t[cc][:, lc * 128 : (lc + 1) * 128],
                        xkvt[:, cc],
                        start=(cc == 0),
                        stop=(cc == 15),
                    )
            kvloc = ph1.tile([128, 4, B * TKV], BF16, tag="kvloc", name="kvloc")
            for lc in range(4):
                nc.vector.tensor_copy(kvloc[:, lc], kvp[lc][:])
            bounce_in = dram.tile([128, 4, B * TKV], BF16)
            bounce_out = dram.tile([NCORES, 128, 4, B, TKV], BF16)
            nc.gpsimd.dma_start(bounce_in[:], kvloc[:])
            nc.gpsimd.collective_compute(
                "AllGather",
                mybir.AluOpType.bypass,
                replica_groups=[list(range(NCORES))],
                ins=[bounce_in[:].opt()],
                outs=[bounce_out[:].opt()],
            )
            kvT = []
            for b in range(B):
                kvb = pers.tile([128, 4, T], BF16, tag=f"kvT{b}", name=f"kvT{b}")
                for lc in range(4):
                    nc.gpsimd.dma_start(
                        kvb[:, lc].rearrange("p (core t) -> p core t", core=NCORES),
                        bounce_out[:, :, lc, b, :].rearrange("core p t -> p core t"),
                    )
                kvT.append(kvb)
            ph1.release()  # wdkv/xkv/kvloc space reused by phase-2 pools

            # ======== phase 2: q projections for all 8 chunks ========
            xpool = ctx.enter_context(tc.tile_pool(name="xp", bufs=2))
            opool = ctx.enter_context(tc.tile_pool(name="op", bufs=2))
            qtpool = ctx.enter_context(tc.tile_pool(name="qt", bufs=1))
            sb2 = ctx.enter_context(tc.tile_pool(name="sb2", bufs=2))
            sb4 = ctx.enter_context(tc.tile_pool(name="sb4", bufs=4))
            sb6 = ctx.enter_context(tc.tile_pool(name="sb6", bufs=6))

            chunks = [(b, j) for b in range(B) for j in range(NT)]
            xtc_tiles = {}

            def load_xtc(idx):
                if idx >= len(chunks):
                    return
                b, j = chunks[idx]
                t0 = j * 512
                xtc = xpool.tile([128, 16, 512], BF16, tag="xtc", name="xtc")
                nc.sync.dma_start(
                    xtc[:],
                    xt_ap[b, :, t0 : t0 + 512].rearrange("(cc p) t -> p cc t", p=128),
                )
                xtc_tiles[idx] = xtc

            load_xtc(0)
            load_xtc(1)

            qts = {}
            for ci, (b, j) in enumerate(chunks):
                load_xtc(ci + 1)
                xtc = xtc_tiles.pop(ci)
                qps = [
                    psB.tile([128, 512], F32, tag="acc2", name=f"qps{h}")
                    for h in range(HPC)
                ]
                for cc in range(16):
                    for h in range(HPC):
                        nc.tensor.matmul(
                            qps[h][:],
                            wq_t[cc][:, h * S : (h + 1) * S],
                            xtc[:, cc],
                            start=(cc == 0),
                            stop=(cc == 15),
                        )
                pair = []
                for h in range(HPC):
                    qt = qtpool.tile([128, 512], BF16, tag=f"qT{ci}{h}", name="qt")
                    nc.vector.tensor_copy(qt[:], qps[h][:])
                    pair.append(qt)
                qts[(b, j)] = pair

            # ======== phase 3: keff + v (all chunks), then attention ========
            vsb = {}
            keff = {}
            for b in range(B):
                vsb[b] = pers.tile(
                    [128, T // 128, HPC * S], BF16, tag=f"vsb{b}", name=f"vsb{b}"
                )
                for h in range(HPC):
                    keff[(b, h)] = pers.tile(
                        [128, T], BF16, tag=f"keff{b}{h}", name=f"keff{b}{h}"
                    )

            for b in range(B):
                for j in range(NT):
                    t0 = j * 512
                    for h in range(HPC):
                        kp = psB.tile([128, 512], F32, tag="acc2", name="kp")
                        for lc in range(4):
                            nc.tensor.matmul(
                                kp[:],
                                wukT[:, lc, h * S : (h + 1) * S],
                                kvT[b][:, lc, t0 : t0 + 512],
                                start=(lc == 0),
                                stop=(lc == 3),
                            )
                        nc.vector.tensor_copy(keff[(b, h)][:, t0 : t0 + 512], kp[:])
                    for tt in range(4):
                        vp = psB.tile([128, HPC * S], F32, tag="acc2", name="vp")
                        for lc in range(4):
                            nc.tensor.matmul(
                                vp[:],
                                kvT[b][:, lc, t0 + tt * 128 : t0 + (tt + 1) * 128],
                                wuv[:, lc, :],
                                start=(lc == 0),
                                stop=(lc == 3),
                            )
                        nc.vector.tensor_copy(vsb[b][:, 4 * j + tt, :], vp[:])

            pending_out = []  # deferred output-projection work items

            def emit_out(item):
                bb, jj, yn_ = item
                tb = jj * 512
                osb = opool.tile([128, 4, 4, 512], BF16, tag="osb", name="osb")
                for tt in range(4):
                    for ncx in range(4):
                        op = psB.tile([128, 512], F32, tag="acc2", name="op")
                        for h in range(HPC):
                            nc.tensor.matmul(
                                op[:],
                                yn_[h][:, tt * 128 : (tt + 1) * 128],
                                wo[:, h, ncx * 512 : (ncx + 1) * 512],
                                start=(h == 0),
                                stop=(h == HPC - 1),
                            )
                        nc.vector.tensor_copy(osb[:, tt, ncx], op[:])
                nc.scalar.dma_start(
                    out_ap[bb, tb : tb + 512, :].rearrange(
                        "(tt p) (ncx f) -> p tt ncx f", p=128, f=512
                    ),
                    osb[:],
                )

            # descending-j order: the tail chunk is a small j=0 one
            att_order = [(b, j) for j in range(NT - 1, -1, -1) for b in range(B)]

            for b, j in att_order:
                nst = 4 * j + 4

                class AttState:
                    pass

                def att_begin(h, qt):
                    st = AttState()
                    st.h = h
                    st.qt = qt
                    st.yps = psB.tile([128, 512], F32, tag="acc2", name="yps")
                    st.dps = psC.tile([1, 512], F32, tag="den", name="dps")
                    st.prev = None
                    return st

                def y_den(st, item):
                    i, n0, ex = item
                    nc.tensor.matmul(
                        st.yps[:, n0:512],
                        vsb[b][:, i, st.h * S : (st.h + 1) * S],
                        ex[:, n0:512],
                        start=(i == 0),
                        stop=(i == nst - 1),
                    )
                    nc.tensor.matmul(
                        st.dps[:, n0:512],
                        onesc[:],
                        ex[:, n0:512],
                        start=(i == 0),
                        stop=(i == nst - 1),
                    )

                def att_steps(st, i_lo, i_hi):
                    for i in range(i_lo, i_hi):
                        n0 = (i - 4 * j) * 128 if i >= 4 * j else 0
                        aps = psA.tile([128, 512], F32, tag="acc4", name="aps")
                        nc.tensor.matmul(
                            aps[:, n0:512],
                            keff[(b, st.h)][:, i * 128 : (i + 1) * 128],
                            st.qt[:, n0:512],
                            start=True,
                            stop=True,
                        )
                        ex = sb6.tile([128, 512], BF16, tag="exp", name="ex")
                        nc.scalar.activation(
                            ex[:, n0:512],
                            aps[:, n0:512],
                            mybir.ActivationFunctionType.Exp,
                            scale=SCALE,
                        )
                        if i >= 4 * j:
                            nc.vector.tensor_mul(
                                ex[:, n0 : n0 + 128],
                                ex[:, n0 : n0 + 128],
                                tri[:],
                            )
                        if st.prev is not None:
                            y_den(st, st.prev)
                        st.prev = (i, n0, ex)

                def att_finish(st):
                    y_den(st, st.prev)
                    rec32 = sb2.tile([1, 512], F32, tag="rec32", name="rec32")
                    nc.vector.reciprocal_approx_fast(rec32[:], st.dps[:])
                    rec = sb2.tile([1, 512], BF16, tag="rec", name="rec")
                    nc.vector.tensor_copy(rec[:], rec32[:])
                    bps = psC.tile([128, 512], F32, tag="den", name="bps")
                    nc.tensor.matmul(bps[:], onesr[:], rec[:], start=True, stop=True)
                    bcs = sb2.tile([128, 512], F32, tag="bcs", name="bcs")
                    nc.vector.tensor_copy(bcs[:], bps[:])
                    y = sb4.tile([128, 512], BF16, tag="yn", name="y")
                    with nc.allow_low_precision(reason="bf16 y for out proj"):
                        nc.vector.tensor_mul(y[:], st.yps[:], bcs[:])
                    return y

                qt0, qt1 = qts[(b, j)]
                st0 = att_begin(0, qt0)
                att_steps(st0, 0, 4 * j)

                # deferred output projection of the previous chunk as filler
                if pending_out:
                    emit_out(pending_out.pop())

                att_steps(st0, 4 * j, nst)
                y0 = att_finish(st0)
                st1 = att_begin(1, qt1)
                att_steps(st1, 0, nst)
                y1 = att_finish(st1)

                pending_out.append((b, j, [y0, y1]))

            emit_out(pending_out.pop())

    nc.compile()
    return nc


def _get_nc():
    if "nc" not in _CACHE:
        _CACHE["nc"] = _build()
    return _CACHE["nc"]


def kernel(x, w_dkv, w_uk, w_uv, w_q, w_o):
    from ml_dtypes import bfloat16

    x = np.asarray(x, dtype=np.float32)
    xT = np.ascontiguousarray(x.transpose(0, 2, 1)).astype(bfloat16)
    w_dkv = np.ascontiguousarray(np.asarray(w_dkv, dtype=np.float32)).astype(bfloat16)
    w_uk = np.asarray(w_uk, dtype=np.float32).astype(bfloat16)
    w_uv = np.asarray(w_uv, dtype=np.float32).astype(bfloat16)
    w_q = np.asarray(w_q, dtype=np.float32).astype(bfloat16)
    w_o = np.asarray(w_o, dtype=np.float32).astype(bfloat16)

    nc = _get_nc()

    tri = np.triu(np.ones((128, 128), dtype=bfloat16))
    ones_col = np.ones((128, 1), dtype=bfloat16)
    ones_row = np.ones((1, 128), dtype=bfloat16)

    in_maps = []
    for c in range(NCORES):
        sl = slice(c * HPC * S, (c + 1) * HPC * S)
        tsl = slice(c * TKV, (c + 1) * TKV)
        in_maps.append(
            {
                "xT": xT,
                "xT_kv": np.ascontiguousarray(xT[:, :, tsl]),
                "w_dkv": w_dkv,
                "w_q_sl": np.ascontiguousarray(w_q[:, sl]),
                "w_ukT_sl": np.ascontiguousarray(w_uk[sl, :].T),
                "w_uv_sl": np.ascontiguousarray(w_uv[:, sl]),
                "w_o_sl": np.ascontiguousarray(w_o[sl, :]),
                "tri": tri,
                "ones_col": ones_col,
                "ones_row": ones_row,
            }
        )

    kwargs = dict(_CACHE.get("run_kwargs", {}))
    res = bass_utils.run_bass_kernel_spmd(
        nc, in_maps, core_ids=list(range(NCORES)), **kwargs
    )
    _CACHE["last_result"] = res

    acc = np.zeros((B, T, C), dtype=np.float64)
    for r in res.results:
        acc += r["out"].astype(np.float64)
    return acc.astype(np.float32)


# revision 16
# speedup vs baseline: 1.1623x; 1.1623x over previous
"""MLA self-attention block (eval mode) on 8 Trainium2 NeuronCores.

Sharding: tensor-parallel over heads (16 heads -> 2 per core), batch kept
whole per core.  The kv-latent projection is T-sharded: each core computes
kv_latent for its own T/8 = 256 token slice (from a per-core xT_kv input
slice), and an on-device AllGather replicates the full [B,T,L] latent to
every core.  Each core computes a partial output through its two heads'
slice of w_o; the host sums the 8 bf16 partials in f32.

Phase structure (the AllGather takes ~55us wall, so it is issued first and
hidden under collective-independent work):
  phase 1: kv partial (64 MMs) -> bounce -> AllGather -> kvT readback
  phase 2: q projections for all 8 (b, t-chunk) pairs (~100us of PE work,
           needs only xT and w_q) while the collective completes
  phase 3: keff + v for all chunks, then attention per chunk in
           DESCENDING j order (tail ends on a 4-step j=0 chunk), with the
           previous chunk's output projection as PE filler

Math (per core, heads h0=2c, h1=2c+1):
  kv_part [L,256]  = w_dkv^T @ xT_kv      (1/8 of the kv FLOPs)
  kvT [L,T]        = AllGather(kv_part)   (bf16, 512KB -> 4MB)
  qT_h [S,T]       = w_q[:,h]^T @ xT      (un-absorbed)
  k_effT [S,T]     = w_uk_h^T @ kvT       (w_uk absorbed into KEYS)
  attT [s,q]       = k_effT^T-tile @ qT   (causal: only s <= q tiles)
  probs            = exp(scale*attT) * tri_mask
  yT [S,q]         = v^T-tiles @ probs    (accumulated over s-tiles)
  den [1,q]        = ones^T @ probs
  out_partial      = (yT/den)^T @ w_o_rows (2 heads accumulated), bf16

All matmuls in bf16 (f32 PSUM accumulate).  xT chunk loads are single
2MB DMAs on the Sync HWDGE queue; wdkv/wq ride the Scalar HWDGE queue
which afterwards carries the per-chunk output stores; the collective
bounce + readback and late weights ride the GpSimd SWDGE queue.
"""

import sys
import os

sys.path.insert(0, "/opt/trn_rl_repo")

import numpy as np
from contextlib import ExitStack

import concourse.bass as bass
import concourse.tile as tile
from concourse import bacc, mybir
from concourse import bass_utils

F32 = mybir.dt.float32
BF16 = mybir.dt.bfloat16

B, T, C = 2, 2048, 2048
H, S, L = 16, 128, 512
NCORES = 8
HPC = H // NCORES  # 2 heads per core
NT = T // 512  # 4 t-chunks of 512
TKV = T // NCORES  # 256-token kv slice per core
SCALE = float(1.0 / np.sqrt(np.float32(C)))

_CACHE = {}


def _build():
    nc = bacc.Bacc("TRN2", target_bir_lowering=False, debug=False, num_devices=NCORES)

    xt_ap = nc.dram_tensor("xT", [B, C, T], BF16, kind="ExternalInput").ap()
    xkv_ap = nc.dram_tensor("xT_kv", [B, C, TKV], BF16, kind="ExternalInput").ap()
    w_dkv = nc.dram_tensor("w_dkv", [C, L], BF16, kind="ExternalInput").ap()
    w_q_sl = nc.dram_tensor("w_q_sl", [C, HPC * S], BF16, kind="ExternalInput").ap()
    w_ukT_sl = nc.dram_tensor("w_ukT_sl", [L, HPC * S], BF16, kind="ExternalInput").ap()
    w_uv_sl = nc.dram_tensor("w_uv_sl", [L, HPC * S], BF16, kind="ExternalInput").ap()
    w_o_sl = nc.dram_tensor("w_o_sl", [HPC * S, C], BF16, kind="ExternalInput").ap()
    tri_d = nc.dram_tensor("tri", [128, 128], BF16, kind="ExternalInput").ap()
    onesc_d = nc.dram_tensor("ones_col", [128, 1], BF16, kind="ExternalInput").ap()
    onesr_d = nc.dram_tensor("ones_row", [1, 128], BF16, kind="ExternalInput").ap()
    out_ap = nc.dram_tensor("out", [B, T, C], BF16, kind="ExternalOutput").ap()

    w_dkv_r = w_dkv.rearrange("(cc p) l -> p cc l", p=128)
    w_q_r = w_q_sl.rearrange("(cc p) f -> p cc f", p=128)

    with tile.TileContext(nc) as tc:
        with ExitStack() as ctx:
            wpool = ctx.enter_context(tc.tile_pool(name="w", bufs=1))
            pers = ctx.enter_context(tc.tile_pool(name="pers", bufs=1))
            psA = ctx.enter_context(tc.tile_pool(name="psA", bufs=4, space="PSUM"))
            psB = ctx.enter_context(tc.tile_pool(name="psB", bufs=3, space="PSUM"))
            psC = ctx.enter_context(tc.tile_pool(name="psC", bufs=1, space="PSUM"))
            dram = ctx.enter_context(tc.tile_pool(name="dram", bufs=1, space="DRAM"))

            # ---- weight loads.  scalar HWDGE: wdkv (kv partial needs it
            #      first), then wq.  sync HWDGE: xkv then the xtc stream.
            #      gpsimd SWDGE: late-phase weights.  Phase-1-only tiles
            #      (wdkv, xkv, kvloc) live in a pool released afterwards. ----
            ph1 = tc.alloc_tile_pool(name="ph1", bufs=1)
            wdkv_t = []
            for cc in range(16):
                wd = ph1.tile([128, L], BF16, tag=f"wdkv{cc}", name=f"wdkv{cc}")
                nc.scalar.dma_start(wd[:], w_dkv_r[:, cc, :])
                wdkv_t.append(wd)
            wq_t = []
            for cc in range(16):
                wqc = wpool.tile([128, HPC * S], BF16, tag=f"wq{cc}", name=f"wq{cc}")
                nc.scalar.dma_start(wqc[:], w_q_r[:, cc, :])
                wq_t.append(wqc)
            xkvt = ph1.tile([128, 16, B, TKV], BF16, tag="xkv", name="xkv")
            for b_ in range(B):
                nc.sync.dma_start(
                    xkvt[:, :, b_, :],
                    xkv_ap[b_].rearrange("(cc p) t -> p cc t", p=128),
                )

            wukT = wpool.tile([128, 4, HPC * S], BF16, tag="wukT", name="wukT")
            nc.gpsimd.dma_start(
                wukT[:], w_ukT_sl.rearrange("(lc p) f -> p lc f", p=128)
            )
            wuv = wpool.tile([128, 4, HPC * S], BF16, tag="wuv", name="wuv")
            nc.gpsimd.dma_start(wuv[:], w_uv_sl.rearrange("(lc p) f -> p lc f", p=128))
            wo = wpool.tile([128, HPC, C], BF16, tag="wo", name="wo")
            nc.gpsimd.dma_start(wo[:], w_o_sl.rearrange("(h p) f -> p h f", p=128))
            tri = wpool.tile([128, 128], BF16, tag="tri", name="tri")
            nc.gpsimd.dma_start(tri[:], tri_d)
            onesc = wpool.tile([128, 1], BF16, tag="onesc", name="onesc")
            nc.gpsimd.dma_start(onesc[:], onesc_d)
            onesr = wpool.tile([1, 128], BF16, tag="onesr", name="onesr")
            nc.gpsimd.dma_start(onesr[:], onesr_d)

            # ======== phase 1: kv partial + AllGather ========
            kvp = [
                psA.tile([128, B * TKV], F32, tag="acc4", name=f"kvp{lc}")
                for lc in range(4)
            ]
            for cc in range(16):
                for lc in range(4):
                    nc.tensor.matmul(
                        kvp[lc][:],
                        wdkv_t[cc][:, lc * 128 : (lc + 1) * 128],
                        xkvt[:, cc],
                        start=(cc == 0),
                        stop=(cc == 15),
                    )
            kvloc = ph1.tile([128, 4, B * TKV], BF16, tag="kvloc", name="kvloc")
            for lc in range(4):
                nc.vector.tensor_copy(kvloc[:, lc], kvp[lc][:])
            bounce_in = dram.tile([128, 4, B * TKV], BF16)
            bounce_out = dram.tile([NCORES, 128, 4, B, TKV], BF16)
            nc.gpsimd.dma_start(bounce_in[:], kvloc[:])
            nc.gpsimd.collective_compute(
                "AllGather",
                mybir.AluOpType.bypass,
                replica_groups=[list(range(NCORES))],
                ins=[bounce_in[:].opt()],
                outs=[bounce_out[:].opt()],
            )
            kvT = []
            for b in range(B):
                kvb = pers.tile([128, 4, T], BF16, tag=f"kvT{b}", name=f"kvT{b}")
                for lc in range(4):
                    nc.gpsimd.dma_start(
                        kvb[:, lc].rearrange("p (core t) -> p core t", core=NCORES),
                        bounce_out[:, :, lc, b, :].rearrange("core p t -> p core t"),
                    )
                kvT.append(kvb)
            ph1.release()  # wdkv/xkv/kvloc space reused by phase-2 pools

            # ======== phase 2: q projections for all 8 chunks ========
            xpool = ctx.enter_context(tc.tile_pool(name="xp", bufs=2))
            opool = ctx.enter_context(tc.tile_pool(name="op", bufs=2))
            qtpool = ctx.enter_context(tc.tile_pool(name="qt", bufs=1))
            sb2 = ctx.enter_context(tc.tile_pool(name="sb2", bufs=2))
            sb4 = ctx.enter_context(tc.tile_pool(name="sb4", bufs=4))
            sb6 = ctx.enter_context(tc.tile_pool(name="sb6", bufs=6))

            chunks = [(b, j) for b in range(B) for j in range(NT)]
            xtc_tiles = {}

            def load_xtc(idx):
                if idx >= len(chunks):
                    return
                b, j = chunks[idx]
                t0 = j * 512
                xtc = xpool.tile([128, 16, 512], BF16, tag="xtc", name="xtc")
                nc.sync.dma_start(
                    xtc[:],
                    xt_ap[b, :, t0 : t0 + 512].rearrange("(cc p) t -> p cc t", p=128),
                )
                xtc_tiles[idx] = xtc

            load_xtc(0)
            load_xtc(1)

            qts = {}
            for ci, (b, j) in enumerate(chunks):
                load_xtc(ci + 1)
                xtc = xtc_tiles.pop(ci)
                qps = [
                    psB.tile([128, 512], F32, tag="acc2", name=f"qps{h}")
                    for h in range(HPC)
                ]
                for cc in range(16):
                    for h in range(HPC):
                        nc.tensor.matmul(
                            qps[h][:],
                            wq_t[cc][:, h * S : (h + 1) * S],
                            xtc[:, cc],
                            start=(cc == 0),
                            stop=(cc == 15),
                        )
                pair = []
                for h in range(HPC):
                    qt = qtpool.tile([128, 512], BF16, tag=f"qT{ci}{h}", name="qt")
                    nc.vector.tensor_copy(qt[:], qps[h][:])
                    pair.append(qt)
                qts[(b, j)] = pair

            # ======== phase 3: keff + v (all chunks), then attention ========
            vsb = {}
            keff = {}
            for b in range(B):
                vsb[b] = pers.tile(
                    [128, T // 128, HPC * S], BF16, tag=f"vsb{b}", name=f"vsb{b}"
                )
                for h in range(HPC):
                    keff[(b, h)] = pers.tile(
                        [128, T], BF16, tag=f"keff{b}{h}", name=f"keff{b}{h}"
                    )

            for b in range(B):
                for j in range(NT):
                    t0 = j * 512
                    for h in range(HPC):
                        kp = psB.tile([128, 512], F32, tag="acc2", name="kp")
                        for lc in range(4):
                            nc.tensor.matmul(
                                kp[:],
                                wukT[:, lc, h * S : (h + 1) * S],
                                kvT[b][:, lc, t0 : t0 + 512],
                                start=(lc == 0),
                                stop=(lc == 3),
                            )
                        nc.vector.tensor_copy(keff[(b, h)][:, t0 : t0 + 512], kp[:])
                    for tt in range(4):
                        vp = psB.tile([128, HPC * S], F32, tag="acc2", name="vp")
                        for lc in range(4):
                            nc.tensor.matmul(
                                vp[:],
                                kvT[b][:, lc, t0 + tt * 128 : t0 + (tt + 1) * 128],
                                wuv[:, lc, :],
                                start=(lc == 0),
                                stop=(lc == 3),
                            )
                        nc.vector.tensor_copy(vsb[b][:, 4 * j + tt, :], vp[:])

            pending_out = []  # deferred output-projection work items

            def emit_out(item):
                bb, jj, yn_ = item
                tb = jj * 512
                osb = opool.tile([128, 4, 4, 512], BF16, tag="osb", name="osb")
                for tt in range(4):
                    for ncx in range(4):
                        op = psB.tile([128, 512], F32, tag="acc2", name="op")
                        for h in range(HPC):
                            nc.tensor.matmul(
                                op[:],
                                yn_[h][:, tt * 128 : (tt + 1) * 128],
                                wo[:, h, ncx * 512 : (ncx + 1) * 512],
                                start=(h == 0),
                                stop=(h == HPC - 1),
                            )
                        nc.vector.tensor_copy(osb[:, tt, ncx], op[:])
                nc.scalar.dma_start(
                    out_ap[bb, tb : tb + 512, :].rearrange(
                        "(tt p) (ncx f) -> p tt ncx f", p=128, f=512
                    ),
                    osb[:],
                )

            # descending-j order: the tail chunk is a small j=0 one
            att_order = [(b, j) for j in range(NT - 1, -1, -1) for b in range(B)]

            for b, j in att_order:
                nst = 4 * j + 4

                class AttState:
                    pass

                def att_begin(h, qt):
                    st = AttState()
                    st.h = h
                    st.qt = qt
                    st.yps = psB.tile([128, 512], F32, tag="acc2", name="yps")
                    st.dps = psC.tile([1, 512], F32, tag="den", name="dps")
                    st.prev = None
                    st.acc = None  # (running bf16 group-sum tile, group n0)
                    st.gidx = 0
                    st.ngroups = (nst + 3) // 4
                    return st

                def flush_den(st):
                    # one denominator matmul per group of <=4 ex tiles
                    acc, gn0 = st.acc
                    nc.tensor.matmul(
                        st.dps[:, gn0:512],
                        onesc[:],
                        acc[:, gn0:512],
                        start=(st.gidx == 0),
                        stop=(st.gidx == st.ngroups - 1),
                    )
                    st.gidx += 1
                    st.acc = None

                def y_den(st, item):
                    i, n0, ex = item
                    nc.tensor.matmul(
                        st.yps[:, n0:512],
                        vsb[b][:, i, st.h * S : (st.h + 1) * S],
                        ex[:, n0:512],
                        start=(i == 0),
                        stop=(i == nst - 1),
                    )
                    # denominator: bf16 group-accumulate on DVE (groups of 4
                    # keep the running-sum precision loss negligible), then a
                    # single ones^T matmul per group
                    if st.acc is None:
                        st.acc = (ex, n0)
                    else:
                        acc, gn0 = st.acc
                        nc.vector.tensor_add(
                            acc[:, n0:512], acc[:, n0:512], ex[:, n0:512]
                        )
                    st.gcount = getattr(st, "gcount", 0) + 1
                    if st.gcount == 4:
                        st.gcount = 0
                        flush_den(st)

                def att_steps(st, i_lo, i_hi):
                    for i in range(i_lo, i_hi):
                        n0 = (i - 4 * j) * 128 if i >= 4 * j else 0
                        aps = psA.tile([128, 512], F32, tag="acc4", name="aps")
                        nc.tensor.matmul(
                            aps[:, n0:512],
                            keff[(b, st.h)][:, i * 128 : (i + 1) * 128],
                            st.qt[:, n0:512],
                            start=True,
                            stop=True,
                        )
                        ex = sb6.tile([128, 512], BF16, tag="exp", name="ex")
                        nc.scalar.activation(
                            ex[:, n0:512],
                            aps[:, n0:512],
                            mybir.ActivationFunctionType.Exp,
                            scale=SCALE,
                        )
                        if i >= 4 * j:
                            nc.vector.tensor_mul(
                                ex[:, n0 : n0 + 128],
                                ex[:, n0 : n0 + 128],
                                tri[:],
                            )
                        if st.prev is not None:
                            y_den(st, st.prev)
                        st.prev = (i, n0, ex)

                def att_finish(st):
                    y_den(st, st.prev)
                    if st.acc is not None:
                        flush_den(st)
                    rec32 = sb2.tile([1, 512], F32, tag="rec32", name="rec32")
                    nc.vector.reciprocal_approx_fast(rec32[:], st.dps[:])
                    rec = sb2.tile([1, 512], BF16, tag="rec", name="rec")
                    nc.vector.tensor_copy(rec[:], rec32[:])
                    bps = psC.tile([128, 512], F32, tag="den", name="bps")
                    nc.tensor.matmul(bps[:], onesr[:], rec[:], start=True, stop=True)
                    bcs = sb2.tile([128, 512], F32, tag="bcs", name="bcs")
                    nc.vector.tensor_copy(bcs[:], bps[:])
                    y = sb4.tile([128, 512], BF16, tag="yn", name="y")
                    with nc.allow_low_precision(reason="bf16 y for out proj"):
                        nc.vector.tensor_mul(y[:], st.yps[:], bcs[:])
                    return y

                qt0, qt1 = qts[(b, j)]
                st0 = att_begin(0, qt0)
                att_steps(st0, 0, 4 * j)

                # deferred output projection of the previous chunk as filler
                if pending_out:
                    emit_out(pending_out.pop())

                att_steps(st0, 4 * j, nst)
                y0 = att_finish(st0)
                st1 = att_begin(1, qt1)
                att_steps(st1, 0, nst)
                y1 = att_finish(st1)

                pending_out.append((b, j, [y0, y1]))

            emit_out(pending_out.pop())

    nc.compile()
    return nc


def _get_nc():
    if "nc" not in _CACHE:
        _CACHE["nc"] = _build()
    return _CACHE["nc"]


def kernel(x, w_dkv, w_uk, w_uv, w_q, w_o):
    from ml_dtypes import bfloat16

    x = np.asarray(x, dtype=np.float32)
    xT = np.ascontiguousarray(x.transpose(0, 2, 1)).astype(bfloat16)
    w_dkv = np.ascontiguousarray(np.asarray(w_dkv, dtype=np.float32)).astype(bfloat16)
    w_uk = np.asarray(w_uk, dtype=np.float32).astype(bfloat16)
    w_uv = np.asarray(w_uv, dtype=np.float32).astype(bfloat16)
    w_q = np.asarray(w_q, dtype=np.float32).astype(bfloat16)
    w_o = np.asarray(w_o, dtype=np.float32).astype(bfloat16)

    nc = _get_nc()

    tri = np.triu(np.ones((128, 128), dtype=bfloat16))
    ones_col = np.ones((128, 1), dtype=bfloat16)
    ones_row = np.ones((1, 128), dtype=bfloat16)

    in_maps = []
    for c in range(NCORES):
        sl = slice(c * HPC * S, (c + 1) * HPC * S)
        tsl = slice(c * TKV, (c + 1) * TKV)
        in_maps.append(
            {
                "xT": xT,
                "xT_kv": np.ascontiguousarray(xT[:, :, tsl]),
                "w_dkv": w_dkv,
                "w_q_sl": np.ascontiguousarray(w_q[:, sl]),
                "w_ukT_sl": np.ascontiguousarray(w_uk[sl, :].T),
                "w_uv_sl": np.ascontiguousarray(w_uv[:, sl]),
                "w_o_sl": np.ascontiguousarray(w_o[sl, :]),
                "tri": tri,
                "ones_col": ones_col,
                "ones_row": ones_row,
            }
        )

    kwargs = dict(_CACHE.get("run_kwargs", {}))
    res = bass_utils.run_bass_kernel_spmd(
        nc, in_maps, core_ids=list(range(NCORES)), **kwargs
    )
    _CACHE["last_result"] = res

    acc = np.zeros((B, T, C), dtype=np.float64)
    for r in res.results:
        acc += r["out"].astype(np.float64)
    return acc.astype(np.float32)


# revision 21
# speedup vs baseline: 1.2325x; 1.0604x over previous
"""MLA self-attention block (eval mode) on 8 Trainium2 NeuronCores.

Sharding: tensor-parallel over heads (16 heads -> 2 per core), batch kept
whole per core.  The kv-latent projection is T-sharded: each core computes
kv_latent for its own T/8 = 256 token slice (from a per-core xT_kv input
slice), and an on-device AllGather replicates the full [B,T,L] latent to
every core.  Each core computes a partial output through its two heads'
slice of w_o; the host sums the 8 bf16 partials in f32.

Phase structure (the AllGather takes ~55us wall, so it is issued first and
hidden under collective-independent work):
  phase 1: kv partial (64 MMs) -> bounce -> AllGather -> kvT readback
  phase 2: q projections for all 8 (b, t-chunk) pairs (~100us of PE work,
           needs only xT and w_q) while the collective completes
  phase 3: keff + v for all chunks, then attention per chunk in
           DESCENDING j order (tail ends on a 4-step j=0 chunk), with the
           previous chunk's output projection as PE filler

Math (per core, heads h0=2c, h1=2c+1):
  kv_part [L,256]  = w_dkv^T @ xT_kv      (1/8 of the kv FLOPs)
  kvT [L,T]        = AllGather(kv_part)   (bf16, 512KB -> 4MB)
  qT_h [S,T]       = w_q[:,h]^T @ xT      (un-absorbed)
  k_effT [S,T]     = w_uk_h^T @ kvT       (w_uk absorbed into KEYS)
  attT [s,q]       = k_effT^T-tile @ qT   (causal: only s <= q tiles)
  probs            = exp(scale*attT) * tri_mask
  yT [S,q]         = v^T-tiles @ probs    (accumulated over s-tiles)
  den [1,q]        = ones^T @ probs
  out_partial      = (yT/den)^T @ w_o_rows (2 heads accumulated), bf16

All matmuls in bf16 (f32 PSUM accumulate).  xT chunk loads are single
2MB DMAs on the Sync HWDGE queue; wdkv/wq ride the Scalar HWDGE queue
which afterwards carries the per-chunk output stores; the collective
bounce + readback and late weights ride the GpSimd SWDGE queue.
"""

import sys
import os

sys.path.insert(0, "/opt/trn_rl_repo")

import numpy as np
from contextlib import ExitStack

import concourse.bass as bass
import concourse.tile as tile
from concourse import bacc, mybir
from concourse import bass_utils

F32 = mybir.dt.float32
BF16 = mybir.dt.bfloat16

B, T, C = 2, 2048, 2048
H, S, L = 16, 128, 512
NCORES = 8
HPC = H // NCORES  # 2 heads per core
NT = T // 512  # 4 t-chunks of 512
TKV = T // NCORES  # 256-token kv slice per core
SCALE = float(1.0 / np.sqrt(np.float32(C)))

_CACHE = {}


def _build():
    nc = bacc.Bacc("TRN2", target_bir_lowering=False, debug=False, num_devices=NCORES)

    # xTq is host-pre-chunked so each (b, t-chunk) DMA reads 16KB contiguous
    # per partition: [b][chunk j][p][cc][t]  (global c = cc*128+p, t = j*512+t)
    xt_ap = nc.dram_tensor("xTq", [B, NT, 128, 16, 512], BF16, kind="ExternalInput").ap()
    xkv_ap = nc.dram_tensor("xT_kv", [128, 16, B, TKV], BF16, kind="ExternalInput").ap()
    w_dkv = nc.dram_tensor("w_dkv", [C, L], BF16, kind="ExternalInput").ap()
    w_q_sl = nc.dram_tensor("w_q_sl", [C, HPC * S], BF16, kind="ExternalInput").ap()
    w_ukT_sl = nc.dram_tensor("w_ukT_sl", [L, HPC * S], BF16, kind="ExternalInput").ap()
    w_uv_sl = nc.dram_tensor("w_uv_sl", [L, HPC * S], BF16, kind="ExternalInput").ap()
    w_o_sl = nc.dram_tensor("w_o_sl", [HPC * S, C], BF16, kind="ExternalInput").ap()
    tri_d = nc.dram_tensor("tri", [128, 128], BF16, kind="ExternalInput").ap()
    onesc_d = nc.dram_tensor("ones_col", [128, 1], BF16, kind="ExternalInput").ap()
    onesr_d = nc.dram_tensor("ones_row", [1, 128], BF16, kind="ExternalInput").ap()
    out_ap = nc.dram_tensor("out", [B, T, C], BF16, kind="ExternalOutput").ap()

    w_dkv_r = w_dkv.rearrange("(cc p) l -> p cc l", p=128)
    w_q_r = w_q_sl.rearrange("(cc p) f -> p cc f", p=128)

    with tile.TileContext(nc) as tc:
        with ExitStack() as ctx:
            wpool = ctx.enter_context(tc.tile_pool(name="w", bufs=1))
            pers = ctx.enter_context(tc.tile_pool(name="pers", bufs=1))
            psA = ctx.enter_context(tc.tile_pool(name="psA", bufs=4, space="PSUM"))
            psB = ctx.enter_context(tc.tile_pool(name="psB", bufs=3, space="PSUM"))
            psC = ctx.enter_context(tc.tile_pool(name="psC", bufs=1, space="PSUM"))
            dram = ctx.enter_context(tc.tile_pool(name="dram", bufs=1, space="DRAM"))

            # ---- weight loads.  scalar HWDGE: wdkv (kv partial needs it
            #      first), then wq.  sync HWDGE: xkv then the xtc stream.
            #      gpsimd SWDGE: late-phase weights.  Phase-1-only tiles
            #      (wdkv, xkv, kvloc) live in a pool released afterwards. ----
            ph1 = tc.alloc_tile_pool(name="ph1", bufs=1)
            wdkv_t = []
            for cc in range(16):
                wd = ph1.tile([128, L], BF16, tag=f"wdkv{cc}", name=f"wdkv{cc}")
                nc.scalar.dma_start(wd[:], w_dkv_r[:, cc, :])
                wdkv_t.append(wd)
            wq_t = []
            for cc in range(16):
                wqc = wpool.tile([128, HPC * S], BF16, tag=f"wq{cc}", name=f"wq{cc}")
                nc.scalar.dma_start(wqc[:], w_q_r[:, cc, :])
                wq_t.append(wqc)
            xkvt = ph1.tile([128, 16, B, TKV], BF16, tag="xkv", name="xkv")
            nc.sync.dma_start(xkvt[:], xkv_ap[:])

            wukT = wpool.tile([128, 4, HPC * S], BF16, tag="wukT", name="wukT")
            nc.gpsimd.dma_start(
                wukT[:], w_ukT_sl.rearrange("(lc p) f -> p lc f", p=128)
            )
            wuv = wpool.tile([128, 4, HPC * S], BF16, tag="wuv", name="wuv")
            nc.gpsimd.dma_start(wuv[:], w_uv_sl.rearrange("(lc p) f -> p lc f", p=128))
            wo = wpool.tile([128, HPC, C], BF16, tag="wo", name="wo")
            nc.gpsimd.dma_start(wo[:], w_o_sl.rearrange("(h p) f -> p h f", p=128))
            tri = wpool.tile([128, 128], BF16, tag="tri", name="tri")
            nc.gpsimd.dma_start(tri[:], tri_d)
            onesc = wpool.tile([128, 1], BF16, tag="onesc", name="onesc")
            nc.gpsimd.dma_start(onesc[:], onesc_d)
            onesr = wpool.tile([1, 128], BF16, tag="onesr", name="onesr")
            nc.gpsimd.dma_start(onesr[:], onesr_d)

            # ======== phase 1: kv partial + AllGather ========
            kvp = [
                psA.tile([128, B * TKV], F32, tag="acc4", name=f"kvp{lc}")
                for lc in range(4)
            ]
            for cc in range(16):
                for lc in range(4):
                    nc.tensor.matmul(
                        kvp[lc][:],
                        wdkv_t[cc][:, lc * 128 : (lc + 1) * 128],
                        xkvt[:, cc],
                        start=(cc == 0),
                        stop=(cc == 15),
                    )
            kvloc = ph1.tile([128, 4, B * TKV], BF16, tag="kvloc", name="kvloc")
            for lc in range(4):
                nc.vector.tensor_copy(kvloc[:, lc], kvp[lc][:])
            bounce_in = dram.tile([128, 4, B * TKV], BF16)
            bounce_out = dram.tile([NCORES, 128, 4, B, TKV], BF16)
            nc.gpsimd.dma_start(bounce_in[:], kvloc[:])
            nc.gpsimd.collective_compute(
                "AllGather",
                mybir.AluOpType.bypass,
                replica_groups=[list(range(NCORES))],
                ins=[bounce_in[:].opt()],
                outs=[bounce_out[:].opt()],
            )
            kvT = []
            for b in range(B):
                kvb = pers.tile([128, 4, T], BF16, tag=f"kvT{b}", name=f"kvT{b}")
                for lc in range(4):
                    nc.gpsimd.dma_start(
                        kvb[:, lc].rearrange("p (core t) -> p core t", core=NCORES),
                        bounce_out[:, :, lc, b, :].rearrange("core p t -> p core t"),
                    )
                kvT.append(kvb)
            ph1.release()  # wdkv/xkv/kvloc space reused by phase-2 pools

            # ======== phase 2: q projections for all 8 chunks ========
            xpool = ctx.enter_context(tc.tile_pool(name="xp", bufs=2))
            opool = ctx.enter_context(tc.tile_pool(name="op", bufs=2))
            qtpool = ctx.enter_context(tc.tile_pool(name="qt", bufs=1))
            sb2 = ctx.enter_context(tc.tile_pool(name="sb2", bufs=2))
            sb4 = ctx.enter_context(tc.tile_pool(name="sb4", bufs=4))
            sb6 = ctx.enter_context(tc.tile_pool(name="sb6", bufs=6))

            chunks = [(b, j) for b in range(B) for j in range(NT)]
            xtc_tiles = {}

            def load_xtc(idx):
                if idx >= len(chunks):
                    return
                b, j = chunks[idx]
                xtc = xpool.tile([128, 16, 512], BF16, tag="xtc", name="xtc")
                eng = nc.sync if idx % 2 == 0 else nc.scalar
                eng.dma_start(xtc[:], xt_ap[b, j])
                xtc_tiles[idx] = xtc

            load_xtc(0)
            load_xtc(1)

            qts = {}
            for ci, (b, j) in enumerate(chunks):
                load_xtc(ci + 1)
                xtc = xtc_tiles.pop(ci)
                qps = [
                    psB.tile([128, 512], F32, tag="acc2", name=f"qps{h}")
                    for h in range(HPC)
                ]
                for cc in range(16):
                    for h in range(HPC):
                        nc.tensor.matmul(
                            qps[h][:],
                            wq_t[cc][:, h * S : (h + 1) * S],
                            xtc[:, cc],
                            start=(cc == 0),
                            stop=(cc == 15),
                        )
                pair = []
                for h in range(HPC):
                    qt = qtpool.tile([128, 512], BF16, tag=f"qT{ci}{h}", name="qt")
                    nc.vector.tensor_copy(qt[:], qps[h][:])
                    pair.append(qt)
                qts[(b, j)] = pair

            # ======== phase 3: keff + v (all chunks), then attention ========
            vsb = {}
            keff = {}
            for b in range(B):
                vsb[b] = pers.tile(
                    [128, T // 128, HPC * S], BF16, tag=f"vsb{b}", name=f"vsb{b}"
                )
                for h in range(HPC):
                    keff[(b, h)] = pers.tile(
                        [128, T], BF16, tag=f"keff{b}{h}", name=f"keff{b}{h}"
                    )

            for b in range(B):
                for j in range(NT):
                    t0 = j * 512
                    for h in range(HPC):
                        kp = psB.tile([128, 512], F32, tag="acc2", name="kp")
                        for lc in range(4):
                            nc.tensor.matmul(
                                kp[:],
                                wukT[:, lc, h * S : (h + 1) * S],
                                kvT[b][:, lc, t0 : t0 + 512],
                                start=(lc == 0),
                                stop=(lc == 3),
                            )
                        nc.vector.tensor_copy(keff[(b, h)][:, t0 : t0 + 512], kp[:])
                    for tt in range(4):
                        vp = psB.tile([128, HPC * S], F32, tag="acc2", name="vp")
                        for lc in range(4):
                            nc.tensor.matmul(
                                vp[:],
                                kvT[b][:, lc, t0 + tt * 128 : t0 + (tt + 1) * 128],
                                wuv[:, lc, :],
                                start=(lc == 0),
                                stop=(lc == 3),
                            )
                        nc.vector.tensor_copy(vsb[b][:, 4 * j + tt, :], vp[:])

            pending_out = []  # deferred output-projection work items

            def emit_out(item):
                bb, jj, yn_ = item
                tb = jj * 512
                osb = opool.tile([128, 4, 4, 512], BF16, tag="osb", name="osb")
                for tt in range(4):
                    for ncx in range(4):
                        op = psB.tile([128, 512], F32, tag="acc2", name="op")
                        for h in range(HPC):
                            nc.tensor.matmul(
                                op[:],
                                yn_[h][:, tt * 128 : (tt + 1) * 128],
                                wo[:, h, ncx * 512 : (ncx + 1) * 512],
                                start=(h == 0),
                                stop=(h == HPC - 1),
                            )
                        nc.vector.tensor_copy(osb[:, tt, ncx], op[:])
                nc.scalar.dma_start(
                    out_ap[bb, tb : tb + 512, :].rearrange(
                        "(tt p) (ncx f) -> p tt ncx f", p=128, f=512
                    ),
                    osb[:],
                )

            # descending-j order: the tail chunk is a small j=0 one
            att_order = [(b, j) for j in range(NT - 1, -1, -1) for b in range(B)]

            for b, j in att_order:
                nst = 4 * j + 4

                class AttState:
                    pass

                def att_begin(h, qt):
                    st = AttState()
                    st.h = h
                    st.qt = qt
                    st.yps = psB.tile([128, 512], F32, tag="acc2", name="yps")
                    st.dps = psC.tile([1, 512], F32, tag="den", name="dps")
                    st.prev = None
                    st.acc = None  # (running bf16 group-sum tile, group n0)
                    st.gidx = 0
                    st.ngroups = (nst + 3) // 4
                    return st

                def flush_den(st):
                    # one denominator matmul per group of <=4 ex tiles
                    acc, gn0 = st.acc
                    nc.tensor.matmul(
                        st.dps[:, gn0:512],
                        onesc[:],
                        acc[:, gn0:512],
                        start=(st.gidx == 0),
                        stop=(st.gidx == st.ngroups - 1),
                    )
                    st.gidx += 1
                    st.acc = None

                def y_den(st, item):
                    i, n0, ex = item
                    nc.tensor.matmul(
                        st.yps[:, n0:512],
                        vsb[b][:, i, st.h * S : (st.h + 1) * S],
                        ex[:, n0:512],
                        start=(i == 0),
                        stop=(i == nst - 1),
                    )
                    # denominator: bf16 group-accumulate on DVE (groups of 4
                    # keep the running-sum precision loss negligible), then a
                    # single ones^T matmul per group
                    if st.acc is None:
                        st.acc = (ex, n0)
                    else:
                        acc, gn0 = st.acc
                        nc.vector.tensor_add(
                            acc[:, n0:512], acc[:, n0:512], ex[:, n0:512]
                        )
                    st.gcount = getattr(st, "gcount", 0) + 1
                    if st.gcount == 4:
                        st.gcount = 0
                        flush_den(st)

                def att_steps(st, i_lo, i_hi):
                    for i in range(i_lo, i_hi):
                        n0 = (i - 4 * j) * 128 if i >= 4 * j else 0
                        aps = psA.tile([128, 512], F32, tag="acc4", name="aps")
                        nc.tensor.matmul(
                            aps[:, n0:512],
                            keff[(b, st.h)][:, i * 128 : (i + 1) * 128],
                            st.qt[:, n0:512],
                            start=True,
                            stop=True,
                        )
                        ex = sb6.tile([128, 512], BF16, tag="exp", name="ex")
                        nc.scalar.activation(
                            ex[:, n0:512],
                            aps[:, n0:512],
                            mybir.ActivationFunctionType.Exp,
                            scale=SCALE,
                        )
                        if i >= 4 * j:
                            nc.vector.tensor_mul(
                                ex[:, n0 : n0 + 128],
                                ex[:, n0 : n0 + 128],
                                tri[:],
                            )
                        if st.prev is not None:
                            y_den(st, st.prev)
                        st.prev = (i, n0, ex)

                def att_finish(st):
                    y_den(st, st.prev)
                    if st.acc is not None:
                        flush_den(st)
                    rec32 = sb2.tile([1, 512], F32, tag="rec32", name="rec32")
                    nc.vector.reciprocal_approx_fast(rec32[:], st.dps[:])
                    rec = sb2.tile([1, 512], BF16, tag="rec", name="rec")
                    nc.vector.tensor_copy(rec[:], rec32[:])
                    bps = psC.tile([128, 512], F32, tag="den", name="bps")
                    nc.tensor.matmul(bps[:], onesr[:], rec[:], start=True, stop=True)
                    bcs = sb2.tile([128, 512], F32, tag="bcs", name="bcs")
                    nc.vector.tensor_copy(bcs[:], bps[:])
                    y = sb4.tile([128, 512], BF16, tag="yn", name="y")
                    with nc.allow_low_precision(reason="bf16 y for out proj"):
                        nc.vector.tensor_mul(y[:], st.yps[:], bcs[:])
                    return y

                qt0, qt1 = qts[(b, j)]
                st0 = att_begin(0, qt0)
                att_steps(st0, 0, 4 * j)

                # deferred output projection of the previous chunk as filler
                if pending_out:
                    emit_out(pending_out.pop())

                att_steps(st0, 4 * j, nst)
                y0 = att_finish(st0)
                st1 = att_begin(1, qt1)
                att_steps(st1, 0, nst)
                y1 = att_finish(st1)

                pending_out.append((b, j, [y0, y1]))

            emit_out(pending_out.pop())

    nc.compile()
    return nc


def _get_nc():
    if "nc" not in _CACHE:
        _CACHE["nc"] = _build()
    return _CACHE["nc"]


def kernel(x, w_dkv, w_uk, w_uv, w_q, w_o):
    from ml_dtypes import bfloat16

    x = np.asarray(x, dtype=np.float32)
    xT = np.ascontiguousarray(x.transpose(0, 2, 1)).astype(bfloat16)  # [B, C, T]
    # [B, NT, 128(p), 16(cc), 512(t)]: 16KB-contiguous per partition per chunk
    xTq = np.ascontiguousarray(
        xT.reshape(B, 16, 128, NT, 512).transpose(0, 3, 2, 1, 4)
    )
    w_dkv = np.ascontiguousarray(np.asarray(w_dkv, dtype=np.float32)).astype(bfloat16)
    w_uk = np.asarray(w_uk, dtype=np.float32).astype(bfloat16)
    w_uv = np.asarray(w_uv, dtype=np.float32).astype(bfloat16)
    w_q = np.asarray(w_q, dtype=np.float32).astype(bfloat16)
    w_o = np.asarray(w_o, dtype=np.float32).astype(bfloat16)

    nc = _get_nc()

    tri = np.triu(np.ones((128, 128), dtype=bfloat16))
    ones_col = np.ones((128, 1), dtype=bfloat16)
    ones_row = np.ones((1, 128), dtype=bfloat16)

    in_maps = []
    for c in range(NCORES):
        sl = slice(c * HPC * S, (c + 1) * HPC * S)
        tsl = slice(c * TKV, (c + 1) * TKV)
        in_maps.append(
            {
                "xTq": xTq,
                "xT_kv": np.ascontiguousarray(
                    xT[:, :, tsl].reshape(B, 16, 128, TKV).transpose(2, 1, 0, 3)
                ),
                "w_dkv": w_dkv,
                "w_q_sl": np.ascontiguousarray(w_q[:, sl]),
                "w_ukT_sl": np.ascontiguousarray(w_uk[sl, :].T),
                "w_uv_sl": np.ascontiguousarray(w_uv[:, sl]),
                "w_o_sl": np.ascontiguousarray(w_o[sl, :]),
                "tri": tri,
                "ones_col": ones_col,
                "ones_row": ones_row,
            }
        )

    kwargs = dict(_CACHE.get("run_kwargs", {}))
    res = bass_utils.run_bass_kernel_spmd(
        nc, in_maps, core_ids=list(range(NCORES)), **kwargs
    )
    _CACHE["last_result"] = res

    acc = np.zeros((B, T, C), dtype=np.float64)
    for r in res.results:
        acc += r["out"].astype(np.float64)
    return acc.astype(np.float32)


# revision 27
# speedup vs baseline: 1.2676x; 1.0284x over previous
"""MLA self-attention block (eval mode) on 8 Trainium2 NeuronCores.

Sharding: tensor-parallel over heads (16 heads -> 2 per core), batch kept
whole per core.  The kv-latent projection is T-sharded: each core computes
kv_latent for its own T/8 = 256 token slice (from a per-core xT_kv input
slice), and an on-device AllGather replicates the full [B,T,L] latent to
every core.  Each core computes a partial output through its two heads'
slice of w_o; the host sums the 8 bf16 partials in f32.

Phase structure (the AllGather takes ~55us wall, so it is issued first and
hidden under collective-independent work):
  phase 1: kv partial (64 MMs) -> bounce -> AllGather -> kvT readback
  phase 2: q projections for all 8 (b, t-chunk) pairs (~100us of PE work,
           needs only xT and w_q) while the collective completes
  phase 3: keff + v for all chunks, then attention per chunk in
           DESCENDING j order (tail ends on a 4-step j=0 chunk), with the
           previous chunk's output projection as PE filler

Math (per core, heads h0=2c, h1=2c+1):
  kv_part [L,256]  = w_dkv^T @ xT_kv      (1/8 of the kv FLOPs)
  kvT [L,T]        = AllGather(kv_part)   (bf16, 512KB -> 4MB)
  qT_h [S,T]       = w_q[:,h]^T @ xT      (un-absorbed)
  k_effT [S,T]     = w_uk_h^T @ kvT       (w_uk absorbed into KEYS)
  attT [s,q]       = k_effT^T-tile @ qT   (causal: only s <= q tiles)
  probs            = exp(scale*attT) * tri_mask
  yT [S,q]         = v^T-tiles @ probs    (accumulated over s-tiles)
  den [1,q]        = ones^T @ probs
  out_partial      = (yT/den)^T @ w_o_rows (2 heads accumulated), bf16

All matmuls in bf16 (f32 PSUM accumulate).  xT chunk loads are single
2MB DMAs on the Sync HWDGE queue; wdkv/wq ride the Scalar HWDGE queue
which afterwards carries the per-chunk output stores; the collective
bounce + readback and late weights ride the GpSimd SWDGE queue.
"""

import sys
import os

sys.path.insert(0, "/opt/trn_rl_repo")

import numpy as np
from contextlib import ExitStack

import concourse.bass as bass
import concourse.tile as tile
from concourse import bacc, mybir
from concourse import bass_utils

F32 = mybir.dt.float32
BF16 = mybir.dt.bfloat16

B, T, C = 2, 2048, 2048
H, S, L = 16, 128, 512
NCORES = 8
HPC = H // NCORES  # 2 heads per core
NT = T // 512  # 4 t-chunks of 512
TKV = T // NCORES  # 256-token kv slice per core
SCALE = float(1.0 / np.sqrt(np.float32(C)))

_CACHE = {}


def _build():
    nc = bacc.Bacc("TRN2", target_bir_lowering=False, debug=False, num_devices=NCORES)

    # xTq is host-pre-chunked so each (b, t-chunk) DMA reads 16KB contiguous
    # per partition: [b][chunk j][p][cc][t]  (global c = cc*128+p, t = j*512+t)
    xt_ap = nc.dram_tensor("xTq", [B, NT, 128, 16, 512], BF16, kind="ExternalInput").ap()
    xkv_ap = nc.dram_tensor("xT_kv", [128, 16, B, TKV], BF16, kind="ExternalInput").ap()
    w_dkv = nc.dram_tensor("w_dkv", [C, L], BF16, kind="ExternalInput").ap()
    w_q_sl = nc.dram_tensor("w_q_sl", [C, HPC * S], BF16, kind="ExternalInput").ap()
    w_ukT_sl = nc.dram_tensor("w_ukT_sl", [L, HPC * S], BF16, kind="ExternalInput").ap()
    w_uv_sl = nc.dram_tensor("w_uv_sl", [L, HPC * S], BF16, kind="ExternalInput").ap()
    w_o_sl = nc.dram_tensor("w_o_sl", [HPC * S, C], BF16, kind="ExternalInput").ap()
    tri_d = nc.dram_tensor("tri", [128, 128], BF16, kind="ExternalInput").ap()
    onesc_d = nc.dram_tensor("ones_col", [128, 1], BF16, kind="ExternalInput").ap()
    onesr_d = nc.dram_tensor("ones_row", [1, 128], BF16, kind="ExternalInput").ap()
    out_ap = nc.dram_tensor("out", [B, T, C], BF16, kind="ExternalOutput").ap()

    w_dkv_r = w_dkv.rearrange("(cc p) l -> p cc l", p=128)
    w_q_r = w_q_sl.rearrange("(cc p) f -> p cc f", p=128)

    with tile.TileContext(nc) as tc:
        with ExitStack() as ctx:
            wpool = ctx.enter_context(tc.tile_pool(name="w", bufs=1))
            pers = ctx.enter_context(tc.tile_pool(name="pers", bufs=1))
            psA = ctx.enter_context(tc.tile_pool(name="psA", bufs=4, space="PSUM"))
            psB = ctx.enter_context(tc.tile_pool(name="psB", bufs=3, space="PSUM"))
            psC = ctx.enter_context(tc.tile_pool(name="psC", bufs=1, space="PSUM"))
            dram = ctx.enter_context(tc.tile_pool(name="dram", bufs=1, space="DRAM"))

            # ---- weight loads.  scalar HWDGE: wdkv (kv partial needs it
            #      first), then wq.  sync HWDGE: xkv then the xtc stream.
            #      gpsimd SWDGE: late-phase weights.  Phase-1-only tiles
            #      (wdkv, xkv, kvloc) live in a pool released afterwards. ----
            ph1 = tc.alloc_tile_pool(name="ph1", bufs=1)
            wdkv_t = []
            for cc in range(16):
                wd = ph1.tile([128, L], BF16, tag=f"wdkv{cc}", name=f"wdkv{cc}")
                nc.scalar.dma_start(wd[:], w_dkv_r[:, cc, :])
                wdkv_t.append(wd)
            wq_t = []
            for cc in range(16):
                wqc = wpool.tile([128, HPC * S], BF16, tag=f"wq{cc}", name=f"wq{cc}")
                nc.scalar.dma_start(wqc[:], w_q_r[:, cc, :])
                wq_t.append(wqc)
            xkvt = ph1.tile([128, 16, B, TKV], BF16, tag="xkv", name="xkv")
            nc.sync.dma_start(xkvt[:], xkv_ap[:])

            wukT = wpool.tile([128, 4, HPC * S], BF16, tag="wukT", name="wukT")
            nc.gpsimd.dma_start(
                wukT[:], w_ukT_sl.rearrange("(lc p) f -> p lc f", p=128)
            )
            wuv = wpool.tile([128, 4, HPC * S], BF16, tag="wuv", name="wuv")
            nc.gpsimd.dma_start(wuv[:], w_uv_sl.rearrange("(lc p) f -> p lc f", p=128))
            wo = wpool.tile([128, HPC, C], BF16, tag="wo", name="wo")
            nc.gpsimd.dma_start(wo[:], w_o_sl.rearrange("(h p) f -> p h f", p=128))
            tri = wpool.tile([128, 128], BF16, tag="tri", name="tri")
            nc.gpsimd.dma_start(tri[:], tri_d)
            onesc = wpool.tile([128, 1], BF16, tag="onesc", name="onesc")
            nc.gpsimd.dma_start(onesc[:], onesc_d)
            onesr = wpool.tile([1, 128], BF16, tag="onesr", name="onesr")
            nc.gpsimd.dma_start(onesr[:], onesr_d)

            # ======== phase 1: kv partial + AllGather ========
            kvp = [
                psA.tile([128, B * TKV], F32, tag="acc4", name=f"kvp{lc}")
                for lc in range(4)
            ]
            for cc in range(16):
                for lc in range(4):
                    nc.tensor.matmul(
                        kvp[lc][:],
                        wdkv_t[cc][:, lc * 128 : (lc + 1) * 128],
                        xkvt[:, cc],
                        start=(cc == 0),
                        stop=(cc == 15),
                    )
            kvloc = ph1.tile([128, 4, B * TKV], BF16, tag="kvloc", name="kvloc")
            for lc in range(4):
                nc.vector.tensor_copy(kvloc[:, lc], kvp[lc][:])
            bounce_in = dram.tile([128, 4, B * TKV], BF16)
            bounce_out = dram.tile([NCORES, 128, 4, B, TKV], BF16)
            nc.gpsimd.dma_start(bounce_in[:], kvloc[:])
            nc.gpsimd.collective_compute(
                "AllGather",
                mybir.AluOpType.bypass,
                replica_groups=[list(range(NCORES))],
                ins=[bounce_in[:].opt()],
                outs=[bounce_out[:].opt()],
            )
            kvT = []
            for b in range(B):
                kvb = pers.tile([128, 4, T], BF16, tag=f"kvT{b}", name=f"kvT{b}")
                for lc in range(4):
                    nc.gpsimd.dma_start(
                        kvb[:, lc].rearrange("p (core t) -> p core t", core=NCORES),
                        bounce_out[:, :, lc, b, :].rearrange("core p t -> p core t"),
                    )
                kvT.append(kvb)
            ph1.release()  # wdkv/xkv/kvloc space reused by phase-2 pools

            # ======== phase 2: q projections for all 8 chunks ========
            xpool = ctx.enter_context(tc.tile_pool(name="xp", bufs=3))
            opool = ctx.enter_context(tc.tile_pool(name="op", bufs=2))
            qtpool = ctx.enter_context(tc.tile_pool(name="qt", bufs=1))
            sb2 = ctx.enter_context(tc.tile_pool(name="sb2", bufs=2))
            sb4 = ctx.enter_context(tc.tile_pool(name="sb4", bufs=4))
            sb6 = ctx.enter_context(tc.tile_pool(name="sb6", bufs=6))

            chunks = [(b, j) for b in range(B) for j in range(NT)]
            xtc_tiles = {}

            def load_xtc(idx):
                if idx >= len(chunks):
                    return
                b, j = chunks[idx]
                xtc = xpool.tile([128, 16, 512], BF16, tag="xtc", name="xtc")
                eng = nc.sync if idx % 2 == 0 else nc.scalar
                eng.dma_start(xtc[:], xt_ap[b, j])
                xtc_tiles[idx] = xtc

            load_xtc(0)
            load_xtc(1)
            load_xtc(2)

            qts = {}
            for ci, (b, j) in enumerate(chunks):
                load_xtc(ci + 3)
                xtc = xtc_tiles.pop(ci)
                qps = [
                    psB.tile([128, 512], F32, tag="acc2", name=f"qps{h}")
                    for h in range(HPC)
                ]
                for cc in range(16):
                    for h in range(HPC):
                        nc.tensor.matmul(
                            qps[h][:],
                            wq_t[cc][:, h * S : (h + 1) * S],
                            xtc[:, cc],
                            start=(cc == 0),
                            stop=(cc == 15),
                        )
                pair = []
                for h in range(HPC):
                    qt = qtpool.tile([128, 512], BF16, tag=f"qT{ci}{h}", name="qt")
                    nc.vector.tensor_copy(qt[:], qps[h][:])
                    pair.append(qt)
                qts[(b, j)] = pair

            # ======== phase 3: keff + v (all chunks), then attention ========
            vsb = {}
            keff = {}
            for b in range(B):
                vsb[b] = pers.tile(
                    [128, T // 128, HPC * S], BF16, tag=f"vsb{b}", name=f"vsb{b}"
                )
                for h in range(HPC):
                    keff[(b, h)] = pers.tile(
                        [128, T], BF16, tag=f"keff{b}{h}", name=f"keff{b}{h}"
                    )

            for b in range(B):
                for j in range(NT):
                    t0 = j * 512
                    for h in range(HPC):
                        kp = psB.tile([128, 512], F32, tag="acc2", name="kp")
                        for lc in range(4):
                            nc.tensor.matmul(
                                kp[:],
                                wukT[:, lc, h * S : (h + 1) * S],
                                kvT[b][:, lc, t0 : t0 + 512],
                                start=(lc == 0),
                                stop=(lc == 3),
                            )
                        nc.scalar.activation(
                            keff[(b, h)][:, t0 : t0 + 512],
                            kp[:],
                            mybir.ActivationFunctionType.Copy,
                        )
                    for tt in range(4):
                        vp = psB.tile([128, HPC * S], F32, tag="acc2", name="vp")
                        for lc in range(4):
                            nc.tensor.matmul(
                                vp[:],
                                kvT[b][:, lc, t0 + tt * 128 : t0 + (tt + 1) * 128],
                                wuv[:, lc, :],
                                start=(lc == 0),
                                stop=(lc == 3),
                            )
                        nc.scalar.activation(
                            vsb[b][:, 4 * j + tt, :],
                            vp[:],
                            mybir.ActivationFunctionType.Copy,
                        )

            pending_out = []  # deferred output-projection work items

            def emit_out(item):
                bb, jj, yn_ = item
                tb = jj * 512
                osb = opool.tile([128, 4, 4, 512], BF16, tag="osb", name="osb")
                for tt in range(4):
                    for ncx in range(4):
                        op = psB.tile([128, 512], F32, tag="acc2", name="op")
                        for h in range(HPC):
                            nc.tensor.matmul(
                                op[:],
                                yn_[h][:, tt * 128 : (tt + 1) * 128],
                                wo[:, h, ncx * 512 : (ncx + 1) * 512],
                                start=(h == 0),
                                stop=(h == HPC - 1),
                            )
                        nc.vector.tensor_copy(osb[:, tt, ncx], op[:])
                nc.scalar.dma_start(
                    out_ap[bb, tb : tb + 512, :].rearrange(
                        "(tt p) (ncx f) -> p tt ncx f", p=128, f=512
                    ),
                    osb[:],
                )

            # descending-j order: the tail chunk is a small j=0 one
            att_order = [(b, j) for j in range(NT - 1, -1, -1) for b in range(B)]

            for b, j in att_order:
                nst = 4 * j + 4

                class AttState:
                    pass

                def att_begin(h, qt):
                    st = AttState()
                    st.h = h
                    st.qt = qt
                    st.yps = psB.tile([128, 512], F32, tag="acc2", name="yps")
                    st.dps = psC.tile([1, 512], F32, tag="den", name="dps")
                    st.prev = None
                    st.acc = None  # (running bf16 group-sum tile, group n0)
                    st.gidx = 0
                    st.ngroups = (nst + 3) // 4
                    return st

                def flush_den(st):
                    # one denominator matmul per group of <=4 ex tiles
                    acc, gn0 = st.acc
                    nc.tensor.matmul(
                        st.dps[:, gn0:512],
                        onesc[:],
                        acc[:, gn0:512],
                        start=(st.gidx == 0),
                        stop=(st.gidx == st.ngroups - 1),
                    )
                    st.gidx += 1
                    st.acc = None

                def y_den(st, item):
                    i, n0, ex = item
                    nc.tensor.matmul(
                        st.yps[:, n0:512],
                        vsb[b][:, i, st.h * S : (st.h + 1) * S],
                        ex[:, n0:512],
                        start=(i == 0),
                        stop=(i == nst - 1),
                    )
                    # denominator: bf16 group-accumulate on DVE (groups of 4
                    # keep the running-sum precision loss negligible), then a
                    # single ones^T matmul per group
                    if st.acc is None:
                        st.acc = (ex, n0)
                    else:
                        acc, gn0 = st.acc
                        nc.vector.tensor_add(
                            acc[:, n0:512], acc[:, n0:512], ex[:, n0:512]
                        )
                    st.gcount = getattr(st, "gcount", 0) + 1
                    if st.gcount == 4:
                        st.gcount = 0
                        flush_den(st)

                def att_steps(st, i_lo, i_hi):
                    for i in range(i_lo, i_hi):
                        n0 = (i - 4 * j) * 128 if i >= 4 * j else 0
                        aps = psA.tile([128, 512], F32, tag="acc4", name="aps")
                        nc.tensor.matmul(
                            aps[:, n0:512],
                            keff[(b, st.h)][:, i * 128 : (i + 1) * 128],
                            st.qt[:, n0:512],
                            start=True,
                            stop=True,
                        )
                        ex = sb6.tile([128, 512], BF16, tag="exp", name="ex")
                        nc.scalar.activation(
                            ex[:, n0:512],
                            aps[:, n0:512],
                            mybir.ActivationFunctionType.Exp,
                            scale=SCALE,
                        )
                        if i >= 4 * j:
                            nc.vector.tensor_mul(
                                ex[:, n0 : n0 + 128],
                                ex[:, n0 : n0 + 128],
                                tri[:],
                            )
                        if st.prev is not None:
                            y_den(st, st.prev)
                        st.prev = (i, n0, ex)

                def att_finish(st):
                    y_den(st, st.prev)
                    if st.acc is not None:
                        flush_den(st)
                    rec32 = sb2.tile([1, 512], F32, tag="rec32", name="rec32")
                    nc.vector.reciprocal_approx_fast(rec32[:], st.dps[:])
                    bcs = sb2.tile([128, 512], F32, tag="bcs", name="bcs")
                    nc.gpsimd.partition_broadcast(bcs[:], rec32[:])
                    y = sb4.tile([128, 512], BF16, tag="yn", name="y")
                    with nc.allow_low_precision(reason="bf16 y for out proj"):
                        nc.vector.tensor_mul(y[:], st.yps[:], bcs[:])
                    return y

                qt0, qt1 = qts[(b, j)]
                st0 = att_begin(0, qt0)
                att_steps(st0, 0, 4 * j)

                # deferred output projection of the previous chunk as filler
                if pending_out:
                    emit_out(pending_out.pop())

                att_steps(st0, 4 * j, nst)
                y0 = att_finish(st0)
                st1 = att_begin(1, qt1)
                att_steps(st1, 0, nst)
                y1 = att_finish(st1)

                pending_out.append((b, j, [y0, y1]))

            emit_out(pending_out.pop())

    nc.compile()
    return nc


def _get_nc():
    if "nc" not in _CACHE:
        _CACHE["nc"] = _build()
    return _CACHE["nc"]


def kernel(x, w_dkv, w_uk, w_uv, w_q, w_o):
    from ml_dtypes import bfloat16

    x = np.asarray(x, dtype=np.float32)
    xT = np.ascontiguousarray(x.transpose(0, 2, 1)).astype(bfloat16)  # [B, C, T]
    # [B, NT, 128(p), 16(cc), 512(t)]: 16KB-contiguous per partition per chunk
    xTq = np.ascontiguousarray(
        xT.reshape(B, 16, 128, NT, 512).transpose(0, 3, 2, 1, 4)
    )
    w_dkv = np.ascontiguousarray(np.asarray(w_dkv, dtype=np.float32)).astype(bfloat16)
    w_uk = np.asarray(w_uk, dtype=np.float32).astype(bfloat16)
    w_uv = np.asarray(w_uv, dtype=np.float32).astype(bfloat16)
    w_q = np.asarray(w_q, dtype=np.float32).astype(bfloat16)
    w_o = np.asarray(w_o, dtype=np.float32).astype(bfloat16)

    nc = _get_nc()

    tri = np.triu(np.ones((128, 128), dtype=bfloat16))
    ones_col = np.ones((128, 1), dtype=bfloat16)
    ones_row = np.ones((1, 128), dtype=bfloat16)

    in_maps = []
    for c in range(NCORES):
        sl = slice(c * HPC * S, (c + 1) * HPC * S)
        tsl = slice(c * TKV, (c + 1) * TKV)
        in_maps.append(
            {
                "xTq": xTq,
                "xT_kv": np.ascontiguousarray(
                    xT[:, :, tsl].reshape(B, 16, 128, TKV).transpose(2, 1, 0, 3)
                ),
                "w_dkv": w_dkv,
                "w_q_sl": np.ascontiguousarray(w_q[:, sl]),
                "w_ukT_sl": np.ascontiguousarray(w_uk[sl, :].T),
                "w_uv_sl": np.ascontiguousarray(w_uv[:, sl]),
                "w_o_sl": np.ascontiguousarray(w_o[sl, :]),
                "tri": tri,
                "ones_col": ones_col,
                "ones_row": ones_row,
            }
        )

    kwargs = dict(_CACHE.get("run_kwargs", {}))
    res = bass_utils.run_bass_kernel_spmd(
        nc, in_maps, core_ids=list(range(NCORES)), **kwargs
    )
    _CACHE["last_result"] = res

    acc = np.zeros((B, T, C), dtype=np.float64)
    for r in res.results:
        acc += r["out"].astype(np.float64)
    return acc.astype(np.float32)
